# revision 1
# baseline (speedup 1.0000x reference)
"""Trainium2 Bass kernel for LocalSpatialSimilarity.

Per sample (B=16, C=256, H=W=64, N=4096 pixels):
  s[p]  = sum_c x[c,p]                (channel sum)
  q[p]  = sum_c x[c,p]^2              (channel sum of squares)
  box   = 3x3 zero-padded box-sum of s (reshaped to 64x64)
  sim   = (box/9 * s) / sqrt(max(q * box^2 * 256/81, 1e-12))
  out   = softmax over p of (mask ? -inf : -sim)
        = (mask ? 0 : exp(-sim)) / total        (sim bounded in [-1,1] -> no
                                                 max-subtraction needed)

Sharding: pure data parallel, 2 samples per core across 8 cores.

On-chip layout: channels on partitions (two 128-chunks), pixels on the free
dim.  Channel reductions are ones-matmuls on the tensor engine into a
[8, 512] PSUM tile (stationary is an indicator column so block j of 512
pixels lands on psum partition j).  Spatial phase runs on a [64 rows,
2 samples, 64 cols] layout where the 3x3 box filter is partition-shifted /
free-shifted adds against a zero-padded tile.
"""

import sys

sys.path.insert(0, "/opt/trn_rl_repo")

import numpy as np

import concourse.bacc as bacc
import concourse.mybir as mybir
import concourse.tile as tile
from concourse.bass_utils import run_bass_kernel_spmd

B, C, H, W = 16, 256, 64, 64
N = H * W
NCORES = 8
SPC = B // NCORES  # samples per core
EPS2 = 1e-12
FP32 = mybir.dt.float32

# float32r: relaxed-precision fp32 matmul, 4x tensor-engine throughput.
MM_DT = mybir.dt.float32r

AF = mybir.ActivationFunctionType
ALU = mybir.AluOpType


def _kernel_body(ctx, tc, x, mask, vband, out, mm_dt=MM_DT, loop=1):
    nc = tc.nc
    HB = 2048  # pixels per spatial half

    consts = ctx.enter_context(tc.tile_pool(name="consts", bufs=1))
    xp = ctx.enter_context(tc.tile_pool(name="xp", bufs=4))
    sqp = ctx.enter_context(tc.tile_pool(name="sqp", bufs=3))
    rows = ctx.enter_context(tc.tile_pool(name="rows", bufs=4))
    single = ctx.enter_context(tc.tile_pool(name="single", bufs=2))
    psa = ctx.enter_context(tc.tile_pool(name="psa", bufs=4, space="PSUM"))
    pss = ctx.enter_context(tc.tile_pool(name="pss", bufs=1, space="PSUM"))

    # Stationary band: D[k, c] = 1 iff c == 7.  Slice [:, 7-j:15-j] is a
    # [128, 8] matrix whose only nonzero column is j, so the ones-matmul
    # lands block j's column sums on psum partition j (zeros elsewhere,
    # accumulated away).
    band = consts.tile([128, 15], FP32)
    nc.vector.memset(band[:], 0.0)
    nc.vector.memset(band[:, 7:8], 1.0)
    ones = consts.tile([128, 64], FP32)
    nc.vector.memset(ones[:], 1.0)
    # Tridiagonal 64x64 ones-band (host-provided): vertical 3-tap box sum as
    # a partition-space matmul (SBUF APs cannot start at unaligned
    # partitions, so partition-shifted adds are not expressible).
    band64 = consts.tile([64, 64], FP32)
    nc.sync.dma_start(out=band64[:], in_=vband.ap())

    for _it in range(loop):
        _one_pass(tc, x, mask, out, band, ones, band64, xp, sqp, rows, single, psa, pss)


def _one_pass(tc, x, mask, out, band, ones, band64, xp, sqp, rows, single, psa, pss):
    nc = tc.nc
    HB = 2048

    # Pair-batched spatial tiles: [row r, sample s, col c].
    Sb = single.tile([64, SPC, 64], FP32, tag="Sb")
    Qt = single.tile([64, SPC, 64], FP32, tag="Qt")

    # Mask, cast bool->f32 during DMA, then scaled to +1e30 ("-inf" additive).
    maskf = single.tile([64, SPC, 64], FP32, tag="maskf")
    nc.gpsimd.dma_start(out=maskf[:], in_=mask.ap().rearrange("s (r c) -> r s c", c=64))
    mb = single.tile([64, SPC, 64], FP32, tag="mb")
    nc.vector.tensor_scalar_mul(mb[:], maskf[:], 1e30)

    for s in range(SPC):
        ps_s = psa.tile([8, 512], FP32, tag="acc")
        ps_q = psa.tile([8, 512], FP32, tag="acc")
        # Whole channel-chunk loads: [128, 4096] with 16 KiB-contiguous rows,
        # alternating between the two HWDGE queues.
        x0 = xp.tile([128, N], FP32, tag="x")
        nc.sync.dma_start(out=x0[:], in_=x[s, 0:128, :])
        x1 = xp.tile([128, N], FP32, tag="x")
        nc.scalar.dma_start(out=x1[:], in_=x[s, 128:256, :])
        # Fold the two channel chunks before the matmul: halves PE work.
        # sf = x0 + x1 (DVE); squares in-place on ACT; qf = x0^2 + x1^2
        # folded in-place into x0's tile (DVE).
        sf = sqp.tile([128, N], FP32, tag="sf")
        nc.vector.tensor_add(sf[:], x0[:], x1[:])
        nc.scalar.activation(x0[:], x0[:], AF.Square)
        nc.scalar.activation(x1[:], x1[:], AF.Square)
        nc.vector.tensor_add(x0[:], x0[:], x1[:])
        for j in range(8):
            st = band[:, 7 - j : 15 - j]
            nc.tensor.matmul(
                ps_s[:],
                st,
                sf[:, 512 * j : 512 * (j + 1)],
                start=j == 0,
                stop=j == 7,
            )
            nc.tensor.matmul(
                ps_q[:],
                st,
                x0[:, 512 * j : 512 * (j + 1)],
                start=j == 0,
                stop=j == 7,
            )
        s_sb = rows.tile([8, 512], FP32, tag="srow")
        q_sb = rows.tile([8, 512], FP32, tag="qrow")
        nc.scalar.copy(s_sb[:], ps_s[:])
        nc.scalar.copy(q_sb[:], ps_q[:])
        # Reshape [8, 512] -> [64, 64]: both APs enumerate pixels in order.
        nc.sync.dma_start(out=Sb[:, s, :], in_=s_sb[:])
        nc.sync.dma_start(out=Qt[:, s, :], in_=q_sb[:])

    # 3x3 box-sum of S with zero padding: vertical 3-tap via tridiagonal
    # matmul over the row-partition dim, horizontal via free-shifted adds.
    v_ps = pss.tile([64, SPC * 64], FP32, tag="vps")
    nc.tensor.matmul(
        v_ps[:], band64[:], Sb[:].rearrange("r s c -> r (s c)"), start=True, stop=True
    )
    Hb = single.tile([64, SPC, 66], FP32)  # cols 0 and 65 stay zero
    nc.vector.memset(Hb[:], 0.0)
    nc.scalar.copy(Hb[:, :, 1:65], v_ps[:].rearrange("r (s c) -> r s c", c=64))
    T1 = single.tile([64, SPC, 64], FP32)
    nc.vector.tensor_add(T1[:], Hb[:, :, 0:64], Hb[:, :, 1:65])
    BOX = single.tile([64, SPC, 64], FP32)
    nc.vector.tensor_add(BOX[:], T1[:], Hb[:, :, 2:66])

    # D = max(box^2 * q * 256/81, eps^2);  R = D^-1/2 via exp(-0.5 ln D)
    # (Rsqrt activation is disallowed for accuracy reasons).
    P = single.tile([64, SPC, 64], FP32)
    nc.vector.tensor_mul(P[:], BOX[:], BOX[:])
    P2 = single.tile([64, SPC, 64], FP32)
    nc.vector.tensor_mul(P2[:], P[:], Qt[:])
    Dt = single.tile([64, SPC, 64], FP32)
    nc.vector.tensor_scalar(
        Dt[:], P2[:], 256.0 / 81.0, EPS2, op0=ALU.mult, op1=ALU.max
    )
    L = single.tile([64, SPC, 64], FP32)
    nc.scalar.activation(L[:], Dt[:], AF.Ln)
    R = single.tile([64, SPC, 64], FP32)
    nc.scalar.activation(R[:], L[:], AF.Exp, scale=-0.5)

    # U = box * s * R;  exp(-(U + 1e30*mask)/9) = masked exp(-sim)
    T = single.tile([64, SPC, 64], FP32)
    nc.vector.tensor_mul(T[:], BOX[:], Sb[:])
    U = single.tile([64, SPC, 64], FP32)
    nc.vector.tensor_mul(U[:], T[:], R[:])
    U2 = single.tile([64, SPC, 64], FP32)
    nc.vector.tensor_add(U2[:], U[:], mb[:])
    EM = single.tile([64, SPC, 64], FP32)
    rowsum = single.tile([64, SPC], FP32)
    for s in range(SPC):
        nc.scalar.activation(
            EM[:, s, :],
            U2[:, s, :],
            AF.Exp,
            scale=-1.0 / 9.0,
            accum_out=rowsum[:, s : s + 1],
        )

    # Per-sample totals: 64->1 ones-matmul, broadcast back 1->64, reciprocal.
    tot_ps = pss.tile([1, SPC], FP32, tag="tot")
    nc.tensor.matmul(tot_ps[:], ones[0:64, 0:1], rowsum[:], start=True, stop=True)
    tots = single.tile([1, SPC], FP32)
    nc.scalar.copy(tots[:], tot_ps[:])
    totb_ps = pss.tile([64, SPC], FP32, tag="totb")
    nc.tensor.matmul(totb_ps[:], ones[0:1, 0:64], tots[:], start=True, stop=True)
    rec = single.tile([64, SPC], FP32)
    nc.vector.reciprocal(rec[:], totb_ps[:])

    OUTt = single.tile([64, SPC, 64], FP32)
    for s in range(SPC):
        nc.vector.tensor_scalar_mul(
            OUTt[:, s, :], EM[:, s, :], rec[:, s : s + 1]
        )
    nc.sync.dma_start(
        out=out.ap().rearrange("s (r c) -> r s c", c=64), in_=OUTt[:]
    )


_NC_CACHE = {}


def _build(mm_dt=MM_DT, loop=1):
    key = (str(mm_dt), loop)
    if key in _NC_CACHE:
        return _NC_CACHE[key]
    nc = bacc.Bacc("TRN2", target_bir_lowering=False, debug=False)
    x = nc.declare_dram_parameter("x", [SPC, C, N], FP32, isOutput=False)
    mask = nc.declare_dram_parameter("mask", [SPC, N], mybir.dt.uint8, isOutput=False)
    vband = nc.declare_dram_parameter("vband", [64, 64], FP32, isOutput=False)
    out = nc.declare_dram_parameter("out", [SPC, N], FP32, isOutput=True)
    from contextlib import ExitStack

    with tile.TileContext(nc) as tc, ExitStack() as ctx:
        _kernel_body(ctx, tc, x, mask, vband, out, mm_dt, loop=loop)
    nc.compile()
    _NC_CACHE[key] = nc
    return nc


def band_matrix() -> np.ndarray:
    idx = np.arange(64)
    return (np.abs(idx[:, None] - idx[None, :]) <= 1).astype(np.float32)


def kernel(x: np.ndarray, prev_drop_mask: np.ndarray) -> np.ndarray:
    nc = _build()
    xs = np.ascontiguousarray(np.asarray(x), dtype=np.float32).reshape(B, C, N)
    ms = np.asarray(prev_drop_mask).astype(np.uint8).reshape(B, N)
    vb = band_matrix()
    in_maps = [
        {
            "x": xs[i * SPC : (i + 1) * SPC],
            "mask": ms[i * SPC : (i + 1) * SPC],
            "vband": vb,
        }
        for i in range(NCORES)
    ]
    res = run_bass_kernel_spmd(nc, in_maps, list(range(NCORES)))
    outs = [res.results[i]["out"] for i in range(NCORES)]
    return np.concatenate(outs, axis=0).reshape(B, H, W)



# revision 8
# speedup vs baseline: 1.2631x; 1.2631x over previous
"""Trainium2 Bass kernel for LocalSpatialSimilarity.

Per sample (B=16, C=256, H=W=64, N=4096 pixels):
  s[p]  = sum_c x[c,p]                (channel sum)
  q[p]  = sum_c x[c,p]^2              (channel sum of squares)
  box   = 3x3 zero-padded box-sum of s (reshaped to 64x64)
  sim   = (box/9 * s) / sqrt(max(q * box^2 * 256/81, 1e-12))
  out   = softmax over p of (mask ? -inf : -sim)
        = (mask ? 0 : exp(-sim)) / total        (sim bounded in [-1,1] -> no
                                                 max-subtraction needed)

Sharding: pure data parallel, 2 samples per core across 8 cores.

v2 design (from trace analysis of v1):
  * fp32r matmuls (4x PE throughput; v1 ran full fp32 and was PE-bound).
  * No DVE channel-folding: both 128-channel chunks stream through the PE
    directly, accumulating in PSUM.  Frees ~18us of DVE time.
  * x loaded in 1 MiB half-chunks alternating across the two HWDGE rings
    so compute starts as soon as the first MiB lands.
  * Channel sums for pixel-block g land on psum partition g via a sliding
    8-wide indicator band (stationary [128,8] slice of a host constant).
  * Spatial phase runs pair-batched on a [128, 64] layout (partition =
    sample*64 + image row) so every DVE/ACT op uses all 128 lanes.
    Vertical 3-tap via block-diagonal tridiagonal matmul, horizontal via
    free-dim shifted adds on a zero-padded tile.
  * Mask is pre-scaled on host (mask * 1e30) and packed into the constant
    tensor; softmax denominators via tiny PE matmuls with indicator
    stationaries.
  * ACT tables (Ln/Exp/Square) warmed by dummy activations at kernel start
    so no table load sits in the epilogue.
  * Small reshape/out DMAs ride the SWDGE (gpsimd) queue so they never
    stall behind the big x loads in the HWDGE FIFOs.
"""

import sys

sys.path.insert(0, "/opt/trn_rl_repo")

import numpy as np

import concourse.bacc as bacc
import concourse.mybir as mybir
import concourse.tile as tile
from concourse.bass_utils import run_bass_kernel_spmd

B, C, H, W = 16, 256, 64, 64
N = H * W
NCORES = 8
SPC = B // NCORES  # samples per core
FP32 = mybir.dt.float32
FP32R = mybir.dt.float32r
BF16 = mybir.dt.bfloat16

AF = mybir.ActivationFunctionType
ALU = mybir.AluOpType

# Const tensor column layout (see const_base()).
CB_BAND = 0       # [:, 0:15]    sliding indicator band
CB_BAND2 = 16     # [:, 16:144]  blockdiag(T64, T64) vertical tridiagonal
CB_SEL = 144      # [:, 144:146] per-sample row selector [128, 2]
CB_SELB = 160     # [0:2, 160:288] broadcast selector [2, 128]
CB_MASK = 320     # [:, 320:384] mask * 1e30 in [128, 64] pair layout
CB_COLS = 384

HC = 2048  # pixels per half-chunk DMA


def _r(ap):
    return ap.bitcast(FP32R)


def _kernel_body(ctx, tc, x, consts, out):
    nc = tc.nc

    cpool = ctx.enter_context(tc.tile_pool(name="consts", bufs=1))
    xp = ctx.enter_context(tc.tile_pool(name="xp", bufs=8))
    sqp = ctx.enter_context(tc.tile_pool(name="sqp", bufs=3))
    rows = ctx.enter_context(tc.tile_pool(name="rows", bufs=4))
    sp = ctx.enter_context(tc.tile_pool(name="sp", bufs=16))
    psa = ctx.enter_context(tc.tile_pool(name="psa", bufs=4, space="PSUM"))
    pss = ctx.enter_context(tc.tile_pool(name="pss", bufs=1, space="PSUM"))

    # Constants + pre-scaled mask in one DMA.  The DMA writes through an
    # f32r-typed AP so the BIR verifier accepts the band slices as rounded
    # fp32r matmul inputs (f32r is bit-identical to f32 on the wire).
    CT = cpool.tile([128, CB_COLS], FP32)
    nc.sync.dma_start(out=_r(CT[:]), in_=_r(consts.ap()))
    # bf16 copy of the sliding band for the sum-of-squares matmuls.
    bandb = cpool.tile([128, 16], BF16)
    nc.vector.tensor_copy(bandb[:, 0:15], CT[:, 0:15])

    # Warm the ACT tables (Ln, Exp, Square) while DMAs fill.
    warm = sp.tile([1, 8], FP32, tag="warm")
    nc.vector.memset(warm[:], 1.0)
    wo = sp.tile([1, 8], FP32, tag="warmout")
    nc.scalar.activation(wo[0:1, 0:2], warm[0:1, 0:2], AF.Ln)
    nc.scalar.activation(wo[0:1, 2:4], warm[0:1, 2:4], AF.Exp)
    nc.scalar.activation(wo[0:1, 4:6], warm[0:1, 4:6], AF.Square)

    # Zero-padded horizontal-shift tile (only cols 0 and 65 must be zero,
    # but a full memset is cheap and runs during the DMA fill).
    Hb = sp.tile([128, 66], FP32, tag="Hb")
    nc.vector.memset(Hb[:], 0.0)

    # All eight 1 MiB x half-chunk loads, k=0 on the sync HWDGE ring and
    # k=1 on the scalar ring, sample 0 first on both.
    xt = {}
    for s in range(SPC):
        for k in range(2):
            for h in range(2):
                t = xp.tile([128, HC], FP32, tag="x")
                eng = nc.sync if k == 0 else nc.scalar
                eng.dma_start(
                    out=_r(t[:]),
                    in_=_r(x[s, 128 * k : 128 * (k + 1), HC * h : HC * (h + 1)]),
                )
                xt[(s, k, h)] = t

    # Channel reductions: sum and sum-of-squares per pixel, [8, 512] psum
    # (row g = pixel block g), then reshaped to the [128, 64] pair layout.
    Sb2 = sp.tile([128, 64], FP32, tag="Sb2")
    Qt2 = sp.tile([128, 64], FP32, tag="Qt2")
    for s in range(SPC):
        ps_s = psa.tile([8, 512], FP32, tag="acc")
        ps_q = psa.tile([8, 512], FP32, tag="acc")
        for k in range(2):
            for h in range(2):
                t = xt[(s, k, h)]
                sq = sqp.tile([128, HC], BF16, tag="sq")
                nc.scalar.activation(sq[:], t[:], AF.Square)
                for l in range(4):
                    g = 4 * h + l
                    st = _r(CT[:, CB_BAND + 7 - g : CB_BAND + 15 - g])
                    stb = bandb[:, 7 - g : 15 - g]
                    first = k == 0 and g == 0
                    last = k == 1 and g == 7
                    nc.tensor.matmul(
                        ps_s[:], st, _r(t[:, 512 * l : 512 * (l + 1)]),
                        start=first, stop=last,
                    )
                    nc.tensor.matmul(
                        ps_q[:], stb, sq[:, 512 * l : 512 * (l + 1)],
                        start=first, stop=last,
                    )
        s_sb = rows.tile([8, 512], FP32, tag="srow")
        q_sb = rows.tile([8, 512], FP32, tag="qrow")
        nc.scalar.copy(s_sb[:], ps_s[:])
        nc.vector.tensor_copy(q_sb[:], ps_q[:])
        # [8, 512] -> [64, 64]: both APs enumerate pixels in order.  SWDGE
        # queue so these never wait behind the big loads in the HWDGE FIFO.
        nc.gpsimd.dma_start(out=Sb2[64 * s : 64 * (s + 1), :], in_=s_sb[:])
        nc.gpsimd.dma_start(out=Qt2[64 * s : 64 * (s + 1), :], in_=q_sb[:])

    # 3x3 box-sum of S: vertical 3-tap via block-diagonal tridiagonal
    # matmul over the partition dim, horizontal via free-shifted adds.
    v_ps = pss.tile([128, 64], FP32, tag="vps")
    nc.tensor.matmul(
        v_ps[:], CT[:, CB_BAND2 : CB_BAND2 + 128], Sb2[:],
        start=True, stop=True,
    )
    nc.scalar.copy(Hb[:, 1:65], v_ps[:])
    T1 = sp.tile([128, 64], FP32)
    nc.vector.tensor_add(T1[:], Hb[:, 0:64], Hb[:, 1:65])
    BOX = sp.tile([128, 64], FP32)
    nc.vector.tensor_add(BOX[:], T1[:], Hb[:, 2:66])

    # sim = (box*s) / sqrt(max(box^2*256/81, eps^2) * q); rsqrt via Ln+Exp
    # (Rsqrt activation is disallowed for accuracy reasons).  The eps clamp
    # rides on box^2 alone: q >= O(100) always, so the product clamp of the
    # reference binds iff this one does (and both only in a regime where
    # sim ~ 0 anyway).
    P = sp.tile([128, 64], FP32)
    nc.scalar.activation(P[:], BOX[:], AF.Square, scale=16.0 / 9.0)
    T = sp.tile([128, 64], FP32)
    nc.vector.tensor_mul(T[:], BOX[:], Sb2[:])
    Dt = sp.tile([128, 64], FP32)
    nc.vector.scalar_tensor_tensor(
        Dt[:], P[:], 1e-12, Qt2[:], op0=ALU.max, op1=ALU.mult
    )
    L = sp.tile([128, 64], FP32)
    nc.scalar.activation(L[:], Dt[:], AF.Ln)
    R = sp.tile([128, 64], FP32)
    nc.scalar.activation(R[:], L[:], AF.Exp, scale=-0.5)

    # U = box*s*R; EM = exp(-(U + 1e30*mask)/9) = masked exp(-sim), with the
    # per-row sums accumulated for free by the ACT op.
    U = sp.tile([128, 64], FP32)
    nc.vector.tensor_mul(U[:], T[:], R[:])
    U2 = sp.tile([128, 64], FP32)
    nc.vector.tensor_add(U2[:], U[:], CT[:, CB_MASK : CB_MASK + 64])
    EM = sp.tile([128, 64], FP32)
    rowsum = sp.tile([128, 1], FP32)
    nc.scalar.activation(
        EM[:], U2[:], AF.Exp, scale=-1.0 / 9.0, accum_out=rowsum[:]
    )

    # Per-sample totals and broadcast back, via tiny indicator matmuls.
    tot_ps = pss.tile([2, 1], FP32, tag="tot")
    nc.tensor.matmul(
        tot_ps[:], CT[:, CB_SEL : CB_SEL + 2], rowsum[:], start=True, stop=True
    )
    rec = sp.tile([2, 1], FP32)
    nc.vector.reciprocal(rec[:], tot_ps[:])
    recb_ps = pss.tile([128, 1], FP32, tag="recb")
    nc.tensor.matmul(
        recb_ps[:], CT[0:2, CB_SELB : CB_SELB + 128], rec[:], start=True, stop=True
    )
    OUTt = sp.tile([128, 64], FP32)
    nc.vector.tensor_scalar_mul(OUTt[:], EM[:], recb_ps[:, 0:1])
    nc.gpsimd.dma_start(
        out=out.ap().rearrange("s (r c) -> (s r) c", c=64), in_=OUTt[:]
    )


_NC_CACHE = {}


def _build():
    key = "v2"
    if key in _NC_CACHE:
        return _NC_CACHE[key]
    nc = bacc.Bacc("TRN2", target_bir_lowering=False, debug=False)
    x = nc.declare_dram_parameter("x", [SPC, C, N], FP32, isOutput=False)
    consts = nc.declare_dram_parameter("consts", [128, CB_COLS], FP32, isOutput=False)
    out = nc.declare_dram_parameter("out", [SPC, N], FP32, isOutput=True)
    from contextlib import ExitStack

    with tile.TileContext(nc) as tc, ExitStack() as ctx:
        _kernel_body(ctx, tc, x, consts, out)
    nc.compile()
    _NC_CACHE[key] = nc
    return nc


def const_base() -> np.ndarray:
    ct = np.zeros((128, CB_COLS), dtype=np.float32)
    # Sliding indicator band: column 7 all-ones; slice [:, 7-g:15-g] puts
    # the ones-column at position g.
    ct[:, CB_BAND + 7] = 1.0
    # Block-diagonal tridiagonal for the vertical 3-tap (both samples).
    idx = np.arange(64)
    t64 = (np.abs(idx[:, None] - idx[None, :]) <= 1).astype(np.float32)
    ct[0:64, CB_BAND2 : CB_BAND2 + 64] = t64
    ct[64:128, CB_BAND2 + 64 : CB_BAND2 + 128] = t64
    # Per-sample selectors.
    ct[0:64, CB_SEL] = 1.0
    ct[64:128, CB_SEL + 1] = 1.0
    ct[0, CB_SELB : CB_SELB + 64] = 1.0
    ct[1, CB_SELB + 64 : CB_SELB + 128] = 1.0
    return ct


_CT_BASE = const_base()


def kernel(x: np.ndarray, prev_drop_mask: np.ndarray) -> np.ndarray:
    nc = _build()
    xs = np.ascontiguousarray(np.asarray(x), dtype=np.float32).reshape(B, C, N)
    mb = (np.asarray(prev_drop_mask).astype(np.float32) * 1e30).reshape(B, H, W)
    in_maps = []
    for i in range(NCORES):
        ct = _CT_BASE.copy()
        ct[0:64, CB_MASK : CB_MASK + 64] = mb[2 * i]
        ct[64:128, CB_MASK : CB_MASK + 64] = mb[2 * i + 1]
        in_maps.append({"x": xs[i * SPC : (i + 1) * SPC], "consts": ct})
    res = run_bass_kernel_spmd(nc, in_maps, list(range(NCORES)))
    outs = [res.results[i]["out"] for i in range(NCORES)]
    return np.concatenate(outs, axis=0).reshape(B, H, W)


# revision 12
# speedup vs baseline: 1.4009x; 1.1091x over previous
"""Trainium2 Bass kernel for LocalSpatialSimilarity.

Per sample (B=16, C=256, H=W=64, N=4096 pixels):
  s[p]  = sum_c x[c,p]                (channel sum)
  q[p]  = sum_c x[c,p]^2              (channel sum of squares)
  box   = 3x3 zero-padded box-sum of s (reshaped to 64x64)
  sim   = (box/9 * s) / sqrt(max(q * box^2 * 256/81, 1e-12))
  out   = softmax over p of (mask ? -inf : -sim)
        = (mask ? 0 : exp(-sim)) / total        (sim bounded in [-1,1] -> no
                                                 max-subtraction needed)

Sharding: pure data parallel, 2 samples per core across 8 cores.

v3 design (evolved from trace analysis):
  * Channel reductions on the PE: fp32r for sum(x) (loads bitcast to f32r
    end-to-end so the BIR verifier sees rounded producers), bf16 for
    sum(x^2) (the squares are cast to bf16 by their producer for free).
  * Channel sums for pixel-block g land on psum partition g via a sliding
    8-wide indicator band; [8, 512] psum then reshaped by DMA into a
    [128, 64] pair layout (partition = sample*64 + image row).
  * x streams in eight 1 MiB half-chunks: channel chunk 0 on the sync
    HWDGE ring, chunk 1 on the gpsimd SWDGE queue, so the scalar (ACT)
    engine's FIFO is never blocked by DMA issue slots.
  * Squares split between ACT (chunk 0) and DVE (chunk 1) so neither
    engine's queue stalls the PE.
  * rsqrt via DVE Newton iteration (magic-constant seed, 2 rounds) -- no
    Ln activation, so the whole kernel lives in ONE ACT table set
    (exp_and_others: exp/square/copy), loaded once at kernel start.
  * PE priming matmuls during the DMA fill keep the HAM clock gate warm
    (otherwise every matmul runs at 1.2 GHz instead of 2.4).
  * Mask pre-scaled on host (mask * 1e30) and packed into the constant
    tensor; softmax denominators via tiny indicator matmuls.
"""

import sys

sys.path.insert(0, "/opt/trn_rl_repo")

import numpy as np

import concourse.bacc as bacc
import concourse.mybir as mybir
import concourse.tile as tile
from concourse.bass_utils import run_bass_kernel_spmd

B, C, H, W = 16, 256, 64, 64
N = H * W
NCORES = 8
SPC = B // NCORES  # samples per core
FP32 = mybir.dt.float32
FP32R = mybir.dt.float32r
BF16 = mybir.dt.bfloat16
I32 = mybir.dt.int32

AF = mybir.ActivationFunctionType
ALU = mybir.AluOpType

# Const tensor column layout (see const_base()).
CB_BAND = 0       # [:, 0:15]    sliding indicator band
CB_BAND2 = 16     # [:, 16:144]  blockdiag(T64, T64) vertical tridiagonal
CB_SEL = 144      # [:, 144:146] per-sample row selector [128, 2]
CB_SELB = 160     # [0:2, 160:288] broadcast selector [2, 128]
CB_MASK = 320     # [:, 320:384] mask * 1e30 in [128, 64] pair layout
CB_COLS = 384

HC = 2048  # pixels per half-chunk DMA
MAGIC = 0x5F3759DF  # rsqrt bit-trick seed


def _r(ap):
    return ap.bitcast(FP32R)


def _i(ap):
    return ap.bitcast(I32)


def _kernel_body(ctx, tc, x, consts, out):
    nc = tc.nc

    cpool = ctx.enter_context(tc.tile_pool(name="consts", bufs=1))
    xp = ctx.enter_context(tc.tile_pool(name="xp", bufs=8))
    sqp = ctx.enter_context(tc.tile_pool(name="sqp", bufs=4))
    rows = ctx.enter_context(tc.tile_pool(name="rows", bufs=4))
    sp = ctx.enter_context(tc.tile_pool(name="sp", bufs=1))
    psa = ctx.enter_context(tc.tile_pool(name="psa", bufs=4, space="PSUM"))
    pss = ctx.enter_context(tc.tile_pool(name="pss", bufs=1, space="PSUM"))

    # Constants + pre-scaled mask in one DMA (f32r-typed so the verifier
    # accepts the band slices as rounded fp32r matmul inputs).
    CT = cpool.tile([128, CB_COLS], FP32)
    nc.sync.dma_start(out=_r(CT[:]), in_=_r(consts.ap()))

    # All eight 1 MiB x half-chunk loads: k=0 on the sync HWDGE ring,
    # k=1 on the gpsimd SWDGE queue, sample 0 first on both.
    xt = {}
    for s in range(SPC):
        for k in range(2):
            for h in range(2):
                t = xp.tile([128, HC], FP32, tag="x")
                eng = nc.sync if k == 0 else nc.gpsimd
                eng.dma_start(
                    out=_r(t[:]),
                    in_=_r(x[s, 128 * k : 128 * (k + 1), HC * h : HC * (h + 1)]),
                )
                xt[(s, k, h)] = t

    # Warm the single ACT table set (exp_and_others: exp/square/copy).
    warm = sp.tile([1, 4], FP32, tag="warm")
    nc.vector.memset(warm[:], 1.0)
    wo = sp.tile([1, 4], FP32, tag="warmout")
    nc.scalar.activation(wo[0:1, 0:2], warm[0:1, 0:2], AF.Exp)

    # bf16 copy of the sliding band for the sum-of-squares matmuls.
    bandb = cpool.tile([128, 16], BF16)
    nc.vector.tensor_copy(bandb[:, 0:15], CT[:, 0:15])

    # Zero-padded horizontal-shift tile.
    Hb = sp.tile([128, 66], FP32, tag="Hb")
    nc.vector.memset(Hb[:], 0.0)

    # Prime the PE HAM clock gate while DMAs fill: ~3.5us of dummy matmul
    # activity lifts the PE from 1.2 to 2.4 GHz before the real work lands.
    prime_ps = pss.tile([8, 384], FP32, tag="prime")
    for i in range(9):
        nc.tensor.matmul(
            prime_ps[:], _r(CT[:, 0:8]), _r(CT[:, 0:CB_COLS]),
            start=i == 0, stop=i == 8,
        )

    # Channel reductions: sum and sum-of-squares per pixel, [8, 512] psum
    # (row g = pixel block g), then reshaped to the [128, 64] pair layout.
    Sb2 = sp.tile([128, 64], FP32, tag="Sb2")
    Qt2 = sp.tile([128, 64], FP32, tag="Qt2")
    for s in range(SPC):
        ps_s = psa.tile([8, 512], FP32, tag="acc")
        ps_q = psa.tile([8, 512], FP32, tag="acc")
        for k in range(2):
            for h in range(2):
                t = xt[(s, k, h)]
                sq = sqp.tile([128, HC], BF16, tag="sq")
                if k == 0:
                    nc.scalar.activation(sq[:], t[:], AF.Square)
                else:
                    nc.vector.tensor_mul(sq[:], t[:], t[:])
                for l in range(4):
                    g = 4 * h + l
                    st = _r(CT[:, CB_BAND + 7 - g : CB_BAND + 15 - g])
                    stb = bandb[:, 7 - g : 15 - g]
                    first = k == 0 and g == 0
                    last = k == 1 and g == 7
                    nc.tensor.matmul(
                        ps_s[:], st, _r(t[:, 512 * l : 512 * (l + 1)]),
                        start=first, stop=last,
                    )
                    nc.tensor.matmul(
                        ps_q[:], stb, sq[:, 512 * l : 512 * (l + 1)],
                        start=first, stop=last,
                    )
        s_sb = rows.tile([8, 512], FP32, tag="srow")
        q_sb = rows.tile([8, 512], FP32, tag="qrow")
        nc.scalar.copy(s_sb[:], ps_s[:])
        nc.vector.tensor_copy(q_sb[:], ps_q[:])
        # [8, 512] -> [64, 64]: both APs enumerate pixels in order.  The
        # scalar HWDGE ring carries only these small transfers, so they
        # never queue behind the big loads.
        nc.scalar.dma_start(out=Sb2[64 * s : 64 * (s + 1), :], in_=s_sb[:])
        nc.scalar.dma_start(out=Qt2[64 * s : 64 * (s + 1), :], in_=q_sb[:])

    # 3x3 box-sum of S: vertical 3-tap via block-diagonal tridiagonal
    # matmul over the partition dim, horizontal via free-shifted adds.
    v_ps = pss.tile([128, 64], FP32, tag="vps")
    nc.tensor.matmul(
        v_ps[:], CT[:, CB_BAND2 : CB_BAND2 + 128], Sb2[:],
        start=True, stop=True,
    )
    nc.scalar.copy(Hb[:, 1:65], v_ps[:])
    T1 = sp.tile([128, 64], FP32)
    nc.vector.tensor_add(T1[:], Hb[:, 0:64], Hb[:, 1:65])
    BOX = sp.tile([128, 64], FP32)
    nc.vector.tensor_add(BOX[:], T1[:], Hb[:, 2:66])

    # sim = (box*s) / sqrt(max((16/9*box)^2, 1e-12) * q).  The eps clamp
    # rides on box^2 alone: q >= O(100) always, so the reference's product
    # clamp binds iff this one does (and only where sim ~ 0 anyway).
    P = sp.tile([128, 64], FP32)
    nc.scalar.activation(P[:], BOX[:], AF.Square, scale=16.0 / 9.0)
    T = sp.tile([128, 64], FP32)
    nc.vector.tensor_mul(T[:], BOX[:], Sb2[:])
    Dt = sp.tile([128, 64], FP32)
    nc.vector.scalar_tensor_tensor(
        Dt[:], P[:], 1e-12, Qt2[:], op0=ALU.max, op1=ALU.mult
    )

    # R = Dt^-1/2 via magic-seed Newton (2 rounds, ~5e-6 rel err): no Ln
    # table needed.  y0 = bitcast(MAGIC - (bitcast(Dt) >> 1)).
    sh = sp.tile([128, 64], FP32)
    nc.vector.tensor_scalar(
        _i(sh[:]), _i(Dt[:]), 1, None, op0=ALU.logical_shift_right
    )
    nt = sp.tile([128, 64], FP32)
    nc.vector.tensor_scalar(
        _i(nt[:]), _i(sh[:]), -1, None, op0=ALU.bitwise_xor
    )
    y = sp.tile([128, 64], FP32)
    nc.vector.tensor_scalar(
        _i(y[:]), _i(nt[:]), MAGIC + 1, None, op0=ALU.add
    )
    for r in range(2):
        a = sp.tile([128, 64], FP32, tag=f"nwt_a{r}")
        nc.vector.tensor_mul(a[:], y[:], y[:])
        hh = sp.tile([128, 64], FP32, tag=f"nwt_h{r}")
        nc.vector.scalar_tensor_tensor(
            hh[:], Dt[:], 0.5, a[:], op0=ALU.mult, op1=ALU.mult
        )
        m1 = sp.tile([128, 64], FP32, tag=f"nwt_m{r}")
        nc.vector.scalar_tensor_tensor(
            m1[:], hh[:], -1.0, y[:], op0=ALU.mult, op1=ALU.mult
        )
        y2 = sp.tile([128, 64], FP32, tag=f"nwt_y{r}")
        nc.vector.scalar_tensor_tensor(
            y2[:], y[:], 1.5, m1[:], op0=ALU.mult, op1=ALU.add
        )
        y = y2

    # U = box*s*R; EM = exp(-(U + 1e30*mask)/9) = masked exp(-sim), with
    # per-row sums accumulated for free by the ACT op.
    U = sp.tile([128, 64], FP32)
    nc.vector.tensor_mul(U[:], T[:], y[:])
    U2 = sp.tile([128, 64], FP32)
    nc.vector.tensor_add(U2[:], U[:], CT[:, CB_MASK : CB_MASK + 64])
    EM = sp.tile([128, 64], FP32)
    rowsum = sp.tile([128, 1], FP32)
    nc.scalar.activation(
        EM[:], U2[:], AF.Exp, scale=-1.0 / 9.0, accum_out=rowsum[:]
    )

    # Per-sample totals and broadcast back, via tiny indicator matmuls.
    tot_ps = pss.tile([2, 1], FP32, tag="tot")
    nc.tensor.matmul(
        tot_ps[:], CT[:, CB_SEL : CB_SEL + 2], rowsum[:], start=True, stop=True
    )
    rec = sp.tile([2, 1], FP32)
    nc.vector.reciprocal(rec[:], tot_ps[:])
    recb_ps = pss.tile([128, 1], FP32, tag="recb")
    nc.tensor.matmul(
        recb_ps[:], CT[0:2, CB_SELB : CB_SELB + 128], rec[:], start=True, stop=True
    )
    OUTt = sp.tile([128, 64], FP32)
    nc.vector.tensor_scalar_mul(OUTt[:], EM[:], recb_ps[:, 0:1])
    nc.scalar.dma_start(
        out=out.ap().rearrange("s (r c) -> (s r) c", c=64), in_=OUTt[:]
    )


_NC_CACHE = {}


def _build():
    key = "v3"
    if key in _NC_CACHE:
        return _NC_CACHE[key]
    nc = bacc.Bacc("TRN2", target_bir_lowering=False, debug=False)
    x = nc.declare_dram_parameter("x", [SPC, C, N], FP32, isOutput=False)
    consts = nc.declare_dram_parameter("consts", [128, CB_COLS], FP32, isOutput=False)
    out = nc.declare_dram_parameter("out", [SPC, N], FP32, isOutput=True)
    from contextlib import ExitStack

    with tile.TileContext(nc) as tc, ExitStack() as ctx:
        _kernel_body(ctx, tc, x, consts, out)
    nc.compile()
    _NC_CACHE[key] = nc
    return nc


def const_base() -> np.ndarray:
    ct = np.zeros((128, CB_COLS), dtype=np.float32)
    # Sliding indicator band: column 7 all-ones; slice [:, 7-g:15-g] puts
    # the ones-column at position g.
    ct[:, CB_BAND + 7] = 1.0
    # Block-diagonal tridiagonal for the vertical 3-tap (both samples).
    idx = np.arange(64)
    t64 = (np.abs(idx[:, None] - idx[None, :]) <= 1).astype(np.float32)
    ct[0:64, CB_BAND2 : CB_BAND2 + 64] = t64
    ct[64:128, CB_BAND2 + 64 : CB_BAND2 + 128] = t64
    # Per-sample selectors.
    ct[0:64, CB_SEL] = 1.0
    ct[64:128, CB_SEL + 1] = 1.0
    ct[0, CB_SELB : CB_SELB + 64] = 1.0
    ct[1, CB_SELB + 64 : CB_SELB + 128] = 1.0
    return ct


_CT_BASE = const_base()


def kernel(x: np.ndarray, prev_drop_mask: np.ndarray) -> np.ndarray:
    nc = _build()
    xs = np.ascontiguousarray(np.asarray(x), dtype=np.float32).reshape(B, C, N)
    mb = (np.asarray(prev_drop_mask).astype(np.float32) * 1e30).reshape(B, H, W)
    in_maps = []
    for i in range(NCORES):
        ct = _CT_BASE.copy()
        ct[0:64, CB_MASK : CB_MASK + 64] = mb[2 * i]
        ct[64:128, CB_MASK : CB_MASK + 64] = mb[2 * i + 1]
        in_maps.append({"x": xs[i * SPC : (i + 1) * SPC], "consts": ct})
    res = run_bass_kernel_spmd(nc, in_maps, list(range(NCORES)))
    outs = [res.results[i]["out"] for i in range(NCORES)]
    return np.concatenate(outs, axis=0).reshape(B, H, W)


# revision 14
# speedup vs baseline: 1.4925x; 1.0654x over previous
"""Trainium2 Bass kernel for LocalSpatialSimilarity.

Per sample (B=16, C=256, H=W=64, N=4096 pixels):
  s[p]  = sum_c x[c,p]                (channel sum)
  q[p]  = sum_c x[c,p]^2              (channel sum of squares)
  box   = 3x3 zero-padded box-sum of s (reshaped to 64x64)
  sim   = (box/9 * s) / sqrt(max(q * box^2 * 256/81, 1e-12))
  out   = softmax over p of (mask ? -inf : -sim)
        = (mask ? 0 : exp(-sim)) / total        (sim bounded in [-1,1] -> no
                                                 max-subtraction needed)

Sharding: pure data parallel, 2 samples per core across 8 cores.

v4 design (evolved through perfetto-trace iterations):
  * x streams in eight 1 MiB half-chunks over BOTH HWDGE rings (channel
    chunk 0 on sync, chunk 1 on scalar) -- measured ~430 GB/s aggregate.
    SWDGE proved ~150 GB/s, so it only carries the tiny reshape/out DMAs
    (where it avoids FIFO-queueing behind the big loads).
  * Channel reductions on the PE: fp32r for sum(x) (DMA writes through
    f32r-typed APs so the BIR verifier sees rounded producers), bf16 for
    sum(x^2).  Sliding 8-wide indicator band stationaries land pixel
    block g on psum partition g; s-matmuls and q-matmuls are emitted in
    separate bursts so a pending square never blocks ready s-matmuls in
    the PE FIFO.
  * Squares split between ACT (chunk 0) and DVE (chunk 1) so neither
    queue stalls, and so DMA-issue slots never sit behind a 2us square.
  * ~5us of dummy bf16 priming matmuls (on a memset tile, no DMA dep)
    warm the PE HAM clock gate before real work lands (cold PE = 1.2 GHz).
  * Spatial phase PER SAMPLE on a [64, 64] layout: sample 0's epilogue
    hides completely under sample 1's DMA/compute; only sample 1's short
    chain is exposed.  Vertical 3-tap via tridiagonal matmul, horizontal
    via free-dim shifted adds, rsqrt via magic-seed Newton (1 round,
    ~2e-3 max rel err) so the whole kernel uses ONE ACT table set
    (exp_and_others), loaded once at kernel start.
  * Mask pre-scaled on host (mask * 1e30) and packed into the constant
    tensor; softmax denominators via tiny indicator matmuls.
"""

import sys

sys.path.insert(0, "/opt/trn_rl_repo")

import numpy as np

import concourse.bacc as bacc
import concourse.mybir as mybir
import concourse.tile as tile
from concourse.bass_utils import run_bass_kernel_spmd

B, C, H, W = 16, 256, 64, 64
N = H * W
NCORES = 8
SPC = B // NCORES  # samples per core
FP32 = mybir.dt.float32
FP32R = mybir.dt.float32r
BF16 = mybir.dt.bfloat16
I32 = mybir.dt.int32

AF = mybir.ActivationFunctionType
ALU = mybir.AluOpType

# Const tensor column layout (see const_base()).
CB_BAND = 0       # [:, 0:15]    sliding indicator band
CB_BAND2 = 16     # [0:64, 16:80]  tridiagonal T64 for the vertical 3-tap
CB_SEL = 144      # [:, 144:145] ones column (totals reduction)
CB_SELB = 160     # [0:1, 160:224] ones row (broadcast)
CB_MASK = 256     # [0:64, 256:384] mask * 1e30, sample s at cols 256+64s
                  # (base partition 0 so DVE tensor-tensor ops can pair it
                  #  with compute tiles -- walrus requires equal base partitions)
CB_COLS = 384

HC = 2048  # pixels per half-chunk DMA
MAGIC = 0x5F3759DF  # rsqrt bit-trick seed


def _r(ap):
    return ap.bitcast(FP32R)


def _i(ap):
    return ap.bitcast(I32)


def _kernel_body(ctx, tc, x, consts, out):
    nc = tc.nc

    cpool = ctx.enter_context(tc.tile_pool(name="consts", bufs=1))
    xp = ctx.enter_context(tc.tile_pool(name="xp", bufs=8))
    sqp = ctx.enter_context(tc.tile_pool(name="sqp", bufs=4))
    rows = ctx.enter_context(tc.tile_pool(name="rows", bufs=2))
    sp = ctx.enter_context(tc.tile_pool(name="sp", bufs=1))
    psa = ctx.enter_context(tc.tile_pool(name="psa", bufs=2, space="PSUM"))
    pss = ctx.enter_context(tc.tile_pool(name="pss", bufs=1, space="PSUM"))

    # Constants + pre-scaled mask in one DMA (f32r-typed so the verifier
    # accepts the band slices as rounded fp32r matmul inputs).
    CT = cpool.tile([128, CB_COLS], FP32)
    nc.sync.dma_start(out=_r(CT[:]), in_=_r(consts.ap()))

    # All eight 1 MiB x half-chunk loads: k=0 on the sync HWDGE ring,
    # k=1 on the scalar ring, sample 0 first on both.
    xt = {}
    for s in range(SPC):
        for k in range(2):
            for h in range(2):
                t = xp.tile([128, HC], FP32, tag="x")
                eng = nc.sync if k == 0 else nc.scalar
                eng.dma_start(
                    out=_r(t[:]),
                    in_=_r(x[s, 128 * k : 128 * (k + 1), HC * h : HC * (h + 1)]),
                )
                xt[(s, k, h)] = t

    # Warm the single ACT table set (exp_and_others: exp/square/copy).
    warm = sp.tile([1, 4], FP32, tag="warm")
    nc.vector.memset(warm[:], 1.0)
    wo = sp.tile([1, 4], FP32, tag="warmout")
    nc.scalar.activation(wo[0:1, 0:2], warm[0:1, 0:2], AF.Exp)

    # bf16 copy of the sliding band for the sum-of-squares matmuls.
    bandb = cpool.tile([128, 16], BF16)
    nc.vector.tensor_copy(bandb[:, 0:15], CT[:, 0:15])

    # Zero-padded horizontal-shift tiles, one per sample.
    Hb = []
    for s in range(SPC):
        hbt = sp.tile([64, 66], FP32, tag=f"Hb{s}")
        nc.vector.memset(hbt[:], 0.0)
        Hb.append(hbt)

    # Prime the PE HAM clock gate while DMAs fill: ~5us of dummy bf16
    # matmul activity lifts the PE from 1.2 to 2.4 GHz before real work.
    pr = cpool.tile([128, 512], BF16)
    nc.gpsimd.memset(pr[:], 0.0)
    prime_ps = pss.tile([8, 512], FP32, tag="prime")
    for i in range(12):
        nc.tensor.matmul(
            prime_ps[:], pr[:, 0:8], pr[:], start=i == 0, stop=i == 11
        )

    # Channel reductions: sum and sum-of-squares per pixel, [8, 512] psum
    # (row g = pixel block g), reshaped to [64, 64] per sample (partition
    # = image row), then the spatial phase runs per sample so sample 0's
    # epilogue hides under sample 1's compute.
    for s in range(SPC):
        ps_s = psa.tile([8, 512], FP32, tag="ps_s")
        ps_q = psa.tile([8, 512], FP32, tag="ps_q")
        for k in range(2):
            for h in range(2):
                t = xt[(s, k, h)]
                sq = sqp.tile([128, HC], BF16, tag="sq")
                if k == 0:
                    nc.scalar.activation(sq[:], t[:], AF.Square)
                else:
                    nc.vector.tensor_mul(sq[:], t[:], t[:])
                for l in range(4):
                    g = 4 * h + l
                    first = k == 0 and g == 0
                    nc.tensor.matmul(
                        ps_s[:],
                        _r(CT[:, CB_BAND + 7 - g : CB_BAND + 15 - g]),
                        _r(t[:, 512 * l : 512 * (l + 1)]),
                        start=first, stop=k == 1 and g == 7,
                    )
                for l in range(4):
                    g = 4 * h + l
                    nc.tensor.matmul(
                        ps_q[:],
                        bandb[:, 7 - g : 15 - g],
                        sq[:, 512 * l : 512 * (l + 1)],
                        start=k == 0 and g == 0, stop=k == 1 and g == 7,
                    )
        s_sb = rows.tile([8, 512], FP32, tag="srow")
        q_sb = rows.tile([8, 512], FP32, tag="qrow")
        nc.scalar.copy(s_sb[:], ps_s[:])
        nc.vector.tensor_copy(q_sb[:], ps_q[:])
        # [8, 512] -> [64, 64]: both APs enumerate pixels in order.  SWDGE
        # queue: never waits behind the big HWDGE loads.
        Sb = sp.tile([64, 64], FP32, tag=f"Sb{s}")
        Qt = sp.tile([64, 64], FP32, tag=f"Qt{s}")
        nc.gpsimd.dma_start(out=Sb[:], in_=s_sb[:])
        nc.gpsimd.dma_start(out=Qt[:], in_=q_sb[:])

        # --- spatial phase for this sample ---
        # Vertical 3-tap: tridiagonal matmul over the row-partition dim.
        v_ps = pss.tile([64, 64], FP32, tag="vps")
        nc.tensor.matmul(
            v_ps[:], CT[0:64, CB_BAND2 : CB_BAND2 + 64], Sb[:],
            start=True, stop=True,
        )
        # Horizontal 3-tap: shifted adds on the zero-padded tile.
        hbt = Hb[s]
        nc.scalar.copy(hbt[:, 1:65], v_ps[:])
        T1 = sp.tile([64, 64], FP32, tag=f"T1_{s}")
        nc.vector.tensor_add(T1[:], hbt[:, 0:64], hbt[:, 1:65])
        BOX = sp.tile([64, 64], FP32, tag=f"BOX{s}")
        nc.vector.tensor_add(BOX[:], T1[:], hbt[:, 2:66])

        # sim = (box*s) / sqrt(max((16/9*box)^2, 1e-12) * q).  The eps
        # clamp rides on box^2 alone: q >= O(100) always, so the
        # reference's product clamp binds iff this one does (and only
        # where sim ~ 0 anyway).
        P = sp.tile([64, 64], FP32, tag=f"P{s}")
        nc.scalar.activation(P[:], BOX[:], AF.Square, scale=16.0 / 9.0)
        T = sp.tile([64, 64], FP32, tag=f"T_{s}")
        nc.vector.tensor_mul(T[:], BOX[:], Sb[:])
        Dt = sp.tile([64, 64], FP32, tag=f"Dt{s}")
        nc.vector.scalar_tensor_tensor(
            Dt[:], P[:], 1e-12, Qt[:], op0=ALU.max, op1=ALU.mult
        )

        # R = Dt^-1/2 via magic-seed Newton (1 round, ~2e-3 rel err --
        # tolerance is 2e-2): y0 = bitcast(MAGIC - (bitcast(Dt) >> 1)).
        nt = sp.tile([64, 64], FP32, tag=f"nt{s}")
        nc.vector.tensor_scalar(
            _i(nt[:]), _i(Dt[:]), 1, -1,
            op0=ALU.logical_shift_right, op1=ALU.bitwise_xor,
        )
        y0 = sp.tile([64, 64], FP32, tag=f"y0{s}")
        nc.vector.tensor_scalar(
            _i(y0[:]), _i(nt[:]), MAGIC + 1, None, op0=ALU.add
        )
        a = sp.tile([64, 64], FP32, tag=f"nwa{s}")
        nc.vector.tensor_mul(a[:], y0[:], y0[:])
        hh = sp.tile([64, 64], FP32, tag=f"nwh{s}")
        nc.vector.scalar_tensor_tensor(
            hh[:], Dt[:], 0.5, a[:], op0=ALU.mult, op1=ALU.mult
        )
        m1 = sp.tile([64, 64], FP32, tag=f"nwm{s}")
        nc.vector.scalar_tensor_tensor(
            m1[:], hh[:], -1.0, y0[:], op0=ALU.mult, op1=ALU.mult
        )
        y = sp.tile([64, 64], FP32, tag=f"nwy{s}")
        nc.vector.scalar_tensor_tensor(
            y[:], y0[:], 1.5, m1[:], op0=ALU.mult, op1=ALU.add
        )

        # U = box*s*R; EM = exp(-(U + 1e30*mask)/9) = masked exp(-sim),
        # with per-row sums accumulated for free by the ACT op.
        U = sp.tile([64, 64], FP32, tag=f"U{s}")
        nc.vector.tensor_mul(U[:], T[:], y[:])
        U2 = sp.tile([64, 64], FP32, tag=f"U2{s}")
        nc.vector.tensor_add(
            U2[:], U[:], CT[0:64, CB_MASK + 64 * s : CB_MASK + 64 * (s + 1)]
        )
        EM = sp.tile([64, 64], FP32, tag=f"EM{s}")
        rowsum = sp.tile([64, 1], FP32, tag=f"rs{s}")
        nc.scalar.activation(
            EM[:], U2[:], AF.Exp, scale=-1.0 / 9.0, accum_out=rowsum[:]
        )

        # Total and broadcast back via tiny ones matmuls.
        tot_ps = pss.tile([1, 1], FP32, tag="tot")
        nc.tensor.matmul(
            tot_ps[:], CT[0:64, CB_SEL : CB_SEL + 1], rowsum[:],
            start=True, stop=True,
        )
        rec = sp.tile([1, 1], FP32, tag=f"rec{s}")
        nc.vector.reciprocal(rec[:], tot_ps[:])
        recb_ps = pss.tile([64, 1], FP32, tag="recb")
        nc.tensor.matmul(
            recb_ps[:], CT[0:1, CB_SELB : CB_SELB + 64], rec[:],
            start=True, stop=True,
        )
        OUTt = sp.tile([64, 64], FP32, tag=f"OUT{s}")
        nc.vector.tensor_scalar_mul(OUTt[:], EM[:], recb_ps[:, 0:1])
        nc.gpsimd.dma_start(
            out=out.ap().rearrange("s (r c) -> s r c", c=64)[s], in_=OUTt[:]
        )


_NC_CACHE = {}


def _build():
    key = "v4"
    if key in _NC_CACHE:
        return _NC_CACHE[key]
    nc = bacc.Bacc("TRN2", target_bir_lowering=False, debug=False)
    x = nc.declare_dram_parameter("x", [SPC, C, N], FP32, isOutput=False)
    consts = nc.declare_dram_parameter("consts", [128, CB_COLS], FP32, isOutput=False)
    out = nc.declare_dram_parameter("out", [SPC, N], FP32, isOutput=True)
    from contextlib import ExitStack

    with tile.TileContext(nc) as tc, ExitStack() as ctx:
        _kernel_body(ctx, tc, x, consts, out)
    nc.compile()
    _NC_CACHE[key] = nc
    return nc


def const_base() -> np.ndarray:
    ct = np.zeros((128, CB_COLS), dtype=np.float32)
    # Sliding indicator band: column 7 all-ones; slice [:, 7-g:15-g] puts
    # the ones-column at position g.
    ct[:, CB_BAND + 7] = 1.0
    # Tridiagonal T64 for the vertical 3-tap.
    idx = np.arange(64)
    t64 = (np.abs(idx[:, None] - idx[None, :]) <= 1).astype(np.float32)
    ct[0:64, CB_BAND2 : CB_BAND2 + 64] = t64
    # Ones column / row for the softmax total + broadcast.
    ct[0:64, CB_SEL] = 1.0
    ct[0, CB_SELB : CB_SELB + 64] = 1.0
    return ct


_CT_BASE = const_base()


def make_in_maps(x: np.ndarray, prev_drop_mask: np.ndarray) -> list:
    xs = np.ascontiguousarray(np.asarray(x), dtype=np.float32).reshape(B, C, N)
    mb = (np.asarray(prev_drop_mask).astype(np.float32) * 1e30).reshape(B, H, W)
    in_maps = []
    for i in range(NCORES):
        ct = _CT_BASE.copy()
        ct[0:64, CB_MASK : CB_MASK + 64] = mb[2 * i]
        ct[0:64, CB_MASK + 64 : CB_MASK + 128] = mb[2 * i + 1]
        in_maps.append({"x": xs[i * SPC : (i + 1) * SPC], "consts": ct})
    return in_maps


def kernel(x: np.ndarray, prev_drop_mask: np.ndarray) -> np.ndarray:
    nc = _build()
    res = run_bass_kernel_spmd(nc, make_in_maps(x, prev_drop_mask), list(range(NCORES)))
    outs = [res.results[i]["out"] for i in range(NCORES)]
    return np.concatenate(outs, axis=0).reshape(B, H, W)


# revision 15
# speedup vs baseline: 1.5090x; 1.0111x over previous
"""Trainium2 Bass kernel for LocalSpatialSimilarity.

Per sample (B=16, C=256, H=W=64, N=4096 pixels):
  s[p]  = sum_c x[c,p]                (channel sum)
  q[p]  = sum_c x[c,p]^2              (channel sum of squares)
  box   = 3x3 zero-padded box-sum of s (reshaped to 64x64)
  sim   = (box/9 * s) / sqrt(max(q * box^2 * 256/81, 1e-12))
  out   = softmax over p of (mask ? -inf : -sim)
        = (mask ? 0 : exp(-sim)) / total        (sim bounded in [-1,1] -> no
                                                 max-subtraction needed)

Sharding: pure data parallel, 2 samples per core across 8 cores.

v4 design (evolved through perfetto-trace iterations):
  * x streams in eight 1 MiB half-chunks over BOTH HWDGE rings (channel
    chunk 0 on sync, chunk 1 on scalar) -- measured ~430 GB/s aggregate.
    SWDGE proved ~150 GB/s, so it only carries the tiny reshape/out DMAs
    (where it avoids FIFO-queueing behind the big loads).
  * Channel reductions on the PE: fp32r for sum(x) (DMA writes through
    f32r-typed APs so the BIR verifier sees rounded producers), bf16 for
    sum(x^2).  Sliding 8-wide indicator band stationaries land pixel
    block g on psum partition g; s-matmuls and q-matmuls are emitted in
    separate bursts so a pending square never blocks ready s-matmuls in
    the PE FIFO.
  * Squares split between ACT (chunk 0) and DVE (chunk 1) so neither
    queue stalls, and so DMA-issue slots never sit behind a 2us square.
  * ~5us of dummy bf16 priming matmuls (on a memset tile, no DMA dep)
    warm the PE HAM clock gate before real work lands (cold PE = 1.2 GHz).
  * Spatial phase PER SAMPLE on a [64, 64] layout: sample 0's epilogue
    hides completely under sample 1's DMA/compute; only sample 1's short
    chain is exposed.  Vertical 3-tap via tridiagonal matmul, horizontal
    via free-dim shifted adds, rsqrt via magic-seed Newton (1 round,
    ~2e-3 max rel err) so the whole kernel uses ONE ACT table set
    (exp_and_others), loaded once at kernel start.
  * Mask pre-scaled on host (mask * 1e30) and packed into the constant
    tensor; softmax denominators via tiny indicator matmuls.
"""

import sys

sys.path.insert(0, "/opt/trn_rl_repo")

import numpy as np

import concourse.bacc as bacc
import concourse.mybir as mybir
import concourse.tile as tile
from concourse.bass_utils import run_bass_kernel_spmd

B, C, H, W = 16, 256, 64, 64
N = H * W
NCORES = 8
SPC = B // NCORES  # samples per core
FP32 = mybir.dt.float32
FP32R = mybir.dt.float32r
BF16 = mybir.dt.bfloat16
I32 = mybir.dt.int32

AF = mybir.ActivationFunctionType
ALU = mybir.AluOpType

# Const tensor column layout (see const_base()).
CB_BAND = 0       # [:, 0:15]    sliding indicator band
CB_BAND2 = 16     # [0:64, 16:80]  tridiagonal T64 for the vertical 3-tap
CB_SEL = 144      # [:, 144:145] ones column (totals reduction)
CB_SELB = 160     # [0:1, 160:224] ones row (broadcast)
CB_MASK = 256     # [0:64, 256:384] mask * 1e30, sample s at cols 256+64s
                  # (base partition 0 so DVE tensor-tensor ops can pair it
                  #  with compute tiles -- walrus requires equal base partitions)
CB_COLS = 384

HC = 2048  # pixels per half-chunk DMA
MAGIC = 0x5F3759DF  # rsqrt bit-trick seed


def _r(ap):
    return ap.bitcast(FP32R)


def _i(ap):
    return ap.bitcast(I32)


def _kernel_body(ctx, tc, x, consts, out):
    nc = tc.nc

    cpool = ctx.enter_context(tc.tile_pool(name="consts", bufs=1))
    xp = ctx.enter_context(tc.tile_pool(name="xp", bufs=8))
    sqp = ctx.enter_context(tc.tile_pool(name="sqp", bufs=8))
    rows = ctx.enter_context(tc.tile_pool(name="rows", bufs=2))
    sp = ctx.enter_context(tc.tile_pool(name="sp", bufs=1))
    psa = ctx.enter_context(tc.tile_pool(name="psa", bufs=2, space="PSUM"))
    pss = ctx.enter_context(tc.tile_pool(name="pss", bufs=1, space="PSUM"))

    # Constants + pre-scaled mask in one DMA (f32r-typed so the verifier
    # accepts the band slices as rounded fp32r matmul inputs).
    CT = cpool.tile([128, CB_COLS], FP32)
    nc.sync.dma_start(out=_r(CT[:]), in_=_r(consts.ap()))

    # All eight 1 MiB x half-chunk loads: k=0 on the sync HWDGE ring,
    # k=1 on the scalar ring, sample 0 first on both.
    xt = {}
    for s in range(SPC):
        for k in range(2):
            for h in range(2):
                t = xp.tile([128, HC], FP32, tag="x")
                eng = nc.sync if k == 0 else nc.scalar
                eng.dma_start(
                    out=_r(t[:]),
                    in_=_r(x[s, 128 * k : 128 * (k + 1), HC * h : HC * (h + 1)]),
                )
                xt[(s, k, h)] = t

    # Warm the single ACT table set (exp_and_others: exp/square/copy).
    warm = sp.tile([1, 4], FP32, tag="warm")
    nc.vector.memset(warm[:], 1.0)
    wo = sp.tile([1, 4], FP32, tag="warmout")
    nc.scalar.activation(wo[0:1, 0:2], warm[0:1, 0:2], AF.Exp)

    # bf16 copy of the sliding band for the sum-of-squares matmuls.
    bandb = cpool.tile([128, 16], BF16)
    nc.vector.tensor_copy(bandb[:, 0:15], CT[:, 0:15])

    # Zero-padded horizontal-shift tiles, one per sample.
    Hb = []
    for s in range(SPC):
        hbt = sp.tile([64, 66], FP32, tag=f"Hb{s}")
        nc.vector.memset(hbt[:], 0.0)
        Hb.append(hbt)

    # Prime the PE HAM clock gate while DMAs fill: ~5us of dummy bf16
    # matmul activity lifts the PE from 1.2 to 2.4 GHz before real work.
    pr = cpool.tile([128, 512], BF16)
    nc.gpsimd.memset(pr[:], 0.0)
    prime_ps = pss.tile([8, 512], FP32, tag="prime")
    for i in range(12):
        nc.tensor.matmul(
            prime_ps[:], pr[:, 0:8], pr[:], start=i == 0, stop=i == 11
        )

    # Channel reductions: sum and sum-of-squares per pixel, [8, 512] psum
    # (row g = pixel block g), reshaped to [64, 64] per sample (partition
    # = image row), then the spatial phase runs per sample so sample 0's
    # epilogue hides under sample 1's compute.
    for s in range(SPC):
        ps_s = psa.tile([8, 512], FP32, tag="ps_s")
        ps_q = psa.tile([8, 512], FP32, tag="ps_q")
        # Chunks in expected DMA-arrival order (the two rings progress in
        # parallel), so the PE FIFO never stalls on a not-yet-landed piece
        # while a landed one waits behind it.  Each piece's square runs as
        # two 1024-pixel halves on ACT and DVE in parallel.
        for ci, (k, h) in enumerate([(0, 0), (1, 0), (0, 1), (1, 1)]):
            t = xt[(s, k, h)]
            sq = sqp.tile([128, HC], BF16, tag="sq")
            eng_a, eng_b = (nc.scalar, nc.vector) if k == 0 else (nc.vector, nc.scalar)
            if eng_a is nc.scalar:
                nc.scalar.activation(sq[:, 0:1024], t[:, 0:1024], AF.Square)
                nc.vector.tensor_mul(sq[:, 1024:2048], t[:, 1024:2048], t[:, 1024:2048])
            else:
                nc.vector.tensor_mul(sq[:, 0:1024], t[:, 0:1024], t[:, 0:1024])
                nc.scalar.activation(sq[:, 1024:2048], t[:, 1024:2048], AF.Square)
            for l in range(4):
                g = 4 * h + l
                nc.tensor.matmul(
                    ps_s[:],
                    _r(CT[:, CB_BAND + 7 - g : CB_BAND + 15 - g]),
                    _r(t[:, 512 * l : 512 * (l + 1)]),
                    start=ci == 0 and l == 0, stop=ci == 3 and l == 3,
                )
            for l in range(4):
                g = 4 * h + l
                nc.tensor.matmul(
                    ps_q[:],
                    bandb[:, 7 - g : 15 - g],
                    sq[:, 512 * l : 512 * (l + 1)],
                    start=ci == 0 and l == 0, stop=ci == 3 and l == 3,
                )
        s_sb = rows.tile([8, 512], FP32, tag="srow")
        q_sb = rows.tile([8, 512], FP32, tag="qrow")
        nc.scalar.copy(s_sb[:], ps_s[:])
        nc.vector.tensor_copy(q_sb[:], ps_q[:])
        # [8, 512] -> [64, 64]: both APs enumerate pixels in order.  SWDGE
        # queue: never waits behind the big HWDGE loads.
        Sb = sp.tile([64, 64], FP32, tag=f"Sb{s}")
        Qt = sp.tile([64, 64], FP32, tag=f"Qt{s}")
        nc.gpsimd.dma_start(out=Sb[:], in_=s_sb[:])
        nc.gpsimd.dma_start(out=Qt[:], in_=q_sb[:])

        # --- spatial phase for this sample ---
        # Vertical 3-tap: tridiagonal matmul over the row-partition dim.
        v_ps = pss.tile([64, 64], FP32, tag="vps")
        nc.tensor.matmul(
            v_ps[:], CT[0:64, CB_BAND2 : CB_BAND2 + 64], Sb[:],
            start=True, stop=True,
        )
        # Horizontal 3-tap: shifted adds on the zero-padded tile.
        hbt = Hb[s]
        nc.scalar.copy(hbt[:, 1:65], v_ps[:])
        T1 = sp.tile([64, 64], FP32, tag=f"T1_{s}")
        nc.vector.tensor_add(T1[:], hbt[:, 0:64], hbt[:, 1:65])
        BOX = sp.tile([64, 64], FP32, tag=f"BOX{s}")
        nc.vector.tensor_add(BOX[:], T1[:], hbt[:, 2:66])

        # sim = (box*s) / sqrt(max((16/9*box)^2, 1e-12) * q).  The eps
        # clamp rides on box^2 alone: q >= O(100) always, so the
        # reference's product clamp binds iff this one does (and only
        # where sim ~ 0 anyway).
        P = sp.tile([64, 64], FP32, tag=f"P{s}")
        nc.scalar.activation(P[:], BOX[:], AF.Square, scale=16.0 / 9.0)
        T = sp.tile([64, 64], FP32, tag=f"T_{s}")
        nc.vector.tensor_mul(T[:], BOX[:], Sb[:])
        Dt = sp.tile([64, 64], FP32, tag=f"Dt{s}")
        nc.vector.scalar_tensor_tensor(
            Dt[:], P[:], 1e-12, Qt[:], op0=ALU.max, op1=ALU.mult
        )

        # R = Dt^-1/2 via magic-seed Newton (1 round, ~2e-3 rel err --
        # tolerance is 2e-2): y0 = bitcast(MAGIC - (bitcast(Dt) >> 1)).
        nt = sp.tile([64, 64], FP32, tag=f"nt{s}")
        nc.vector.tensor_scalar(
            _i(nt[:]), _i(Dt[:]), 1, -1,
            op0=ALU.logical_shift_right, op1=ALU.bitwise_xor,
        )
        y0 = sp.tile([64, 64], FP32, tag=f"y0{s}")
        nc.vector.tensor_scalar(
            _i(y0[:]), _i(nt[:]), MAGIC + 1, None, op0=ALU.add
        )
        a = sp.tile([64, 64], FP32, tag=f"nwa{s}")
        nc.vector.tensor_mul(a[:], y0[:], y0[:])
        hh = sp.tile([64, 64], FP32, tag=f"nwh{s}")
        nc.vector.scalar_tensor_tensor(
            hh[:], Dt[:], 0.5, a[:], op0=ALU.mult, op1=ALU.mult
        )
        m1 = sp.tile([64, 64], FP32, tag=f"nwm{s}")
        nc.vector.scalar_tensor_tensor(
            m1[:], hh[:], -1.0, y0[:], op0=ALU.mult, op1=ALU.mult
        )
        y = sp.tile([64, 64], FP32, tag=f"nwy{s}")
        nc.vector.scalar_tensor_tensor(
            y[:], y0[:], 1.5, m1[:], op0=ALU.mult, op1=ALU.add
        )

        # U = box*s*R; EM = exp(-(U + 1e30*mask)/9) = masked exp(-sim),
        # with per-row sums accumulated for free by the ACT op.
        U = sp.tile([64, 64], FP32, tag=f"U{s}")
        nc.vector.tensor_mul(U[:], T[:], y[:])
        U2 = sp.tile([64, 64], FP32, tag=f"U2{s}")
        nc.vector.tensor_add(
            U2[:], U[:], CT[0:64, CB_MASK + 64 * s : CB_MASK + 64 * (s + 1)]
        )
        EM = sp.tile([64, 64], FP32, tag=f"EM{s}")
        rowsum = sp.tile([64, 1], FP32, tag=f"rs{s}")
        nc.scalar.activation(
            EM[:], U2[:], AF.Exp, scale=-1.0 / 9.0, accum_out=rowsum[:]
        )

        # Total and broadcast back via tiny ones matmuls.
        tot_ps = pss.tile([1, 1], FP32, tag="tot")
        nc.tensor.matmul(
            tot_ps[:], CT[0:64, CB_SEL : CB_SEL + 1], rowsum[:],
            start=True, stop=True,
        )
        rec = sp.tile([1, 1], FP32, tag=f"rec{s}")
        nc.vector.reciprocal(rec[:], tot_ps[:])
        recb_ps = pss.tile([64, 1], FP32, tag="recb")
        nc.tensor.matmul(
            recb_ps[:], CT[0:1, CB_SELB : CB_SELB + 64], rec[:],
            start=True, stop=True,
        )
        OUTt = sp.tile([64, 64], FP32, tag=f"OUT{s}")
        nc.vector.tensor_scalar_mul(OUTt[:], EM[:], recb_ps[:, 0:1])
        nc.sync.dma_start(
            out=out.ap().rearrange("s (r c) -> s r c", c=64)[s], in_=OUTt[:]
        )


_NC_CACHE = {}


def _build():
    key = "v5"
    if key in _NC_CACHE:
        return _NC_CACHE[key]
    nc = bacc.Bacc("TRN2", target_bir_lowering=False, debug=False)
    x = nc.declare_dram_parameter("x", [SPC, C, N], FP32, isOutput=False)
    consts = nc.declare_dram_parameter("consts", [128, CB_COLS], FP32, isOutput=False)
    out = nc.declare_dram_parameter("out", [SPC, N], FP32, isOutput=True)
    from contextlib import ExitStack

    with tile.TileContext(nc) as tc, ExitStack() as ctx:
        _kernel_body(ctx, tc, x, consts, out)
    nc.compile()
    _NC_CACHE[key] = nc
    return nc


def const_base() -> np.ndarray:
    ct = np.zeros((128, CB_COLS), dtype=np.float32)
    # Sliding indicator band: column 7 all-ones; slice [:, 7-g:15-g] puts
    # the ones-column at position g.
    ct[:, CB_BAND + 7] = 1.0
    # Tridiagonal T64 for the vertical 3-tap.
    idx = np.arange(64)
    t64 = (np.abs(idx[:, None] - idx[None, :]) <= 1).astype(np.float32)
    ct[0:64, CB_BAND2 : CB_BAND2 + 64] = t64
    # Ones column / row for the softmax total + broadcast.
    ct[0:64, CB_SEL] = 1.0
    ct[0, CB_SELB : CB_SELB + 64] = 1.0
    return ct


_CT_BASE = const_base()


def make_in_maps(x: np.ndarray, prev_drop_mask: np.ndarray) -> list:
    xs = np.ascontiguousarray(np.asarray(x), dtype=np.float32).reshape(B, C, N)
    mb = (np.asarray(prev_drop_mask).astype(np.float32) * 1e30).reshape(B, H, W)
    in_maps = []
    for i in range(NCORES):
        ct = _CT_BASE.copy()
        ct[0:64, CB_MASK : CB_MASK + 64] = mb[2 * i]
        ct[0:64, CB_MASK + 64 : CB_MASK + 128] = mb[2 * i + 1]
        in_maps.append({"x": xs[i * SPC : (i + 1) * SPC], "consts": ct})
    return in_maps


def kernel(x: np.ndarray, prev_drop_mask: np.ndarray) -> np.ndarray:
    nc = _build()
    res = run_bass_kernel_spmd(nc, make_in_maps(x, prev_drop_mask), list(range(NCORES)))
    outs = [res.results[i]["out"] for i in range(NCORES)]
    return np.concatenate(outs, axis=0).reshape(B, H, W)
